# revision 1
# baseline (speedup 1.0000x reference)
"""GCN layer (DGL GraphConv norm='both' + relu + residual + LayerNorm) on 8 trn2 NeuronCores.

Strategy: data-parallel over destination nodes. Each core owns N/8 output nodes
and the edges whose dst lands in its range (host partitions/sorts edges by dst).
Each core:
  1. builds an fp16 table h = feats * rsqrt(deg_out) in its DRAM (p-major staging),
  2. per 128-dst-node block: indirect-DMA gathers h[src] rows for the block's
     edges, builds one-hot (edge x dst-slot) indicator tiles on DVE, and
     matmul-accumulates the segment sum in PSUM (feature-major),
  3. applies W (matmul), transposes to node-major, then deg_in scaling, bias,
     relu, residual and LayerNorm, and writes its slice of the output.
No collectives: inputs needed by several cores are replicated via in_maps.
"""

import numpy as np


def _ensure_path():
    try:
        import concourse  # noqa: F401
    except ImportError:
        import sys

        for p in ("/opt/trn_rl_repo", "/root/.axon_site/_ro/trn_rl_repo"):
            if p not in sys.path:
                sys.path.insert(0, p)


P = 128
LN_EPS = 1e-5


# ---------------------------------------------------------------- host prep
def host_prep(feats, src, dst, W, b, gamma, beta, n_cores):
    N, D = feats.shape
    assert N % n_cores == 0
    npc = N // n_cores                      # nodes per core
    nblk = (npc + P - 1) // P               # 128-node blocks per core
    rows_pp = (N + 1 + P - 1) // P          # table rows per partition
    npad = rows_pp * P                      # padded table rows (>= N+1)
    zero_row = N                            # an all-zero table row for padding

    src = np.asarray(src).astype(np.int64)
    dst = np.asarray(dst).astype(np.int64)

    feats_pad = np.zeros((npad, D), np.float32)
    feats_pad[:N] = feats

    order = np.argsort(dst, kind="stable")
    src_s = src[order]
    dst_s = dst[order]
    rp_dst = np.searchsorted(dst_s, np.arange(N + 1)).astype(np.int64)
    rp_src = np.searchsorted(np.sort(src), np.arange(npad + 1)).astype(np.int32)

    # per (core, block) edge counts -> shared tile schedule K[j]
    cnts = np.zeros((n_cores, nblk), np.int64)
    for m in range(n_cores):
        base = m * npc
        loc = rp_dst[base : base + npc + 1]
        lo = loc[np.minimum(np.arange(nblk) * P, npc)]
        hi = loc[np.minimum((np.arange(nblk) + 1) * P, npc)]
        cnts[m] = hi - lo
    K = np.maximum(1, (-(-cnts // P)).max(axis=0)).astype(np.int64)  # [nblk]
    C = np.concatenate([[0], np.cumsum(K)]).astype(np.int64)
    tot_k = int(C[-1])

    gidx = np.full((n_cores, P, tot_k), zero_row, np.int32)
    dstcol = np.full((n_cores, P, tot_k), -1.0, np.float32)
    for m in range(n_cores):
        base = m * npc
        for j in range(nblk):
            e0 = rp_dst[base + min(j * P, npc)]
            cnt = int(cnts[m, j])
            kj = int(K[j])
            bi = np.full(kj * P, zero_row, np.int64)
            bd = np.full(kj * P, -1.0, np.float32)
            bi[:cnt] = src_s[e0 : e0 + cnt]
            bd[:cnt] = dst_s[e0 : e0 + cnt] - (base + j * P)
            gidx[m, :, C[j] : C[j + 1]] = bi.reshape(kj, P).T
            dstcol[m, :, C[j] : C[j + 1]] = bd.reshape(kj, P).T

    # per-core local dst rowptr, padded to nblk*P+1 entries
    rp_dst_mine = np.zeros((n_cores, nblk * P + 1), np.int32)
    for m in range(n_cores):
        base = m * npc
        loc = rp_dst[base : base + npc + 1]
        rp_dst_mine[m, : npc + 1] = loc
        rp_dst_mine[m, npc + 1 :] = loc[-1]

    iota = np.tile(np.arange(P, dtype=np.float16), (P, 1))
    ident = np.eye(D, dtype=np.float32)

    in_maps = []
    for m in range(n_cores):
        base = m * npc
        in_maps.append(
            {
                "feats_pad": feats_pad,
                "feats_mine": np.ascontiguousarray(feats[base : base + npc]).astype(np.float32),
                "rp_src": rp_src,
                "rp_dst_mine": np.ascontiguousarray(rp_dst_mine[m]),
                "gidx": np.ascontiguousarray(gidx[m]),
                "dstcol": np.ascontiguousarray(dstcol[m]),
                "Wmat": np.asarray(W, np.float32),
                "bvec": np.asarray(b, np.float32),
                "gamma": np.asarray(gamma, np.float32),
                "beta": np.asarray(beta, np.float32),
                "iota": iota,
                "ident": ident,
            }
        )

    meta = dict(
        N=N, D=D, n_cores=n_cores, npc=npc, nblk=nblk, rows_pp=rows_pp,
        npad=npad, K=[int(k) for k in K], C=[int(c) for c in C], tot_k=tot_k,
    )
    return in_maps, meta


def _split_multiwaits(nc, mybir):
    """This walrus build allows only one sync-wait per instruction; hoist
    extra waits onto same-engine NoOps placed just before the instruction."""
    n = 0
    for f in nc.m.functions:
        for bb in f.blocks:
            newlist = []
            for inst in bb.instructions:
                si = getattr(inst, "sync_info", None)
                if si is not None and len(si.on_wait) > 1:
                    waits = list(si.on_wait)
                    for w in waits[:-1]:
                        nop = mybir.InstNoOp(name=f"I-WS-{n}", ins=[], outs=[])
                        n += 1
                        nop.engine = inst.engine
                        nop.sync_info = mybir.SyncInfo(on_wait=[w], on_update=[])
                        newlist.append(nop)
                    inst.sync_info = mybir.SyncInfo(
                        on_wait=[waits[-1]], on_update=list(si.on_update)
                    )
                newlist.append(inst)
            bb.instructions = newlist


# ---------------------------------------------------------------- device program
def build_nc(meta, debug=False, split_waits=True):
    _ensure_path()
    from contextlib import ExitStack

    import concourse.bass as bass
    import concourse.tile as tile
    from concourse import mybir

    dt = mybir.dt
    f32, f16, i32 = dt.float32, dt.float16, dt.int32
    Alu = mybir.AluOpType
    Act = mybir.ActivationFunctionType

    N = meta["N"]
    D = meta["D"]
    npc = meta["npc"]
    nblk = meta["nblk"]
    rows_pp = meta["rows_pp"]
    npad = meta["npad"]
    K = meta["K"]
    C = meta["C"]
    tot_k = meta["tot_k"]
    kmax = max(K)

    nc = bass.Bass()

    feats_pad = nc.declare_dram_parameter("feats_pad", [npad, D], f32, isOutput=False)
    feats_mine = nc.declare_dram_parameter("feats_mine", [npc, D], f32, isOutput=False)
    rp_src = nc.declare_dram_parameter("rp_src", [npad + 1], i32, isOutput=False)
    rp_dst_mine = nc.declare_dram_parameter("rp_dst_mine", [nblk * P + 1], i32, isOutput=False)
    gidx_in = nc.declare_dram_parameter("gidx", [P, tot_k], i32, isOutput=False)
    dstcol_in = nc.declare_dram_parameter("dstcol", [P, tot_k], f32, isOutput=False)
    W_in = nc.declare_dram_parameter("Wmat", [D, D], f32, isOutput=False)
    b_in = nc.declare_dram_parameter("bvec", [D], f32, isOutput=False)
    gamma_in = nc.declare_dram_parameter("gamma", [D], f32, isOutput=False)
    beta_in = nc.declare_dram_parameter("beta", [D], f32, isOutput=False)
    iota_in = nc.declare_dram_parameter("iota", [P, P], f16, isOutput=False)
    ident_in = nc.declare_dram_parameter("ident", [D, D], f32, isOutput=False)
    out_t = nc.declare_dram_parameter("out", [npc, D], f32, isOutput=True)

    h_dram = nc.dram_tensor("h_table", [npad, D], f16)
    din_scr = nc.dram_tensor("din_scr", [nblk * P], f32)
    if debug:
        k0 = K[0]
        dbg_h = nc.declare_dram_parameter("dbg_h", [npad, D], f16, isOutput=True)
        dbg_din = nc.declare_dram_parameter("dbg_din", [nblk * P], f32, isOutput=True)
        dbg_g = nc.declare_dram_parameter("dbg_g", [P, k0 * D], f16, isOutput=True)
        dbg_agg = nc.declare_dram_parameter("dbg_agg", [D, P], f16, isOutput=True)
        dbg_deg = nc.declare_dram_parameter("dbg_deg", [P, rows_pp], f32, isOutput=True)

    def bcast_row(ap, parts):
        # [L] dram vector -> [parts, L] partition-broadcast access pattern
        return bass.AP(tensor=ap.tensor, offset=ap.offset, ap=[[0, parts]] + list(ap.ap))

    with ExitStack() as ctx:
        tc = ctx.enter_context(tile.TileContext(nc))
        const = ctx.enter_context(tc.tile_pool(name="const", bufs=1))
        ph1 = ctx.enter_context(tc.tile_pool(name="ph1", bufs=2))
        gp = ctx.enter_context(tc.tile_pool(name="gp", bufs=12))
        indp = ctx.enter_context(tc.tile_pool(name="indp", bufs=8))
        ep = ctx.enter_context(tc.tile_pool(name="ep", bufs=3))
        pp = ctx.enter_context(tc.tile_pool(name="pp", bufs=2, space="PSUM"))

        # ---- constants -------------------------------------------------
        iota_sb = const.tile([P, P], f16)
        nc.sync.dma_start(out=iota_sb[:], in_=iota_in[:])
        ident_sb = const.tile([D, D], f32)
        nc.sync.dma_start(out=ident_sb[:], in_=ident_in[:])
        w_f32 = const.tile([D, D], f32)
        nc.sync.dma_start(out=w_f32[:], in_=W_in[:])
        w_sb = const.tile([D, D], f16)
        nc.vector.tensor_copy(out=w_sb[:], in_=w_f32[:])
        b_bc = const.tile([P, D], f32)
        nc.sync.dma_start(out=b_bc[:], in_=bcast_row(b_in[:], P))
        gamma_bc = const.tile([P, D], f32)
        nc.sync.dma_start(out=gamma_bc[:], in_=bcast_row(gamma_in[:], P))
        beta_bc = const.tile([P, D], f32)
        nc.sync.dma_start(out=beta_bc[:], in_=bcast_row(beta_in[:], P))
        eps_sb = const.tile([P, 1], f32)
        nc.vector.memset(eps_sb[:], LN_EPS)
        gidx_sb = const.tile([P, tot_k], i32)
        nc.sync.dma_start(out=gidx_sb[:], in_=gidx_in[:])
        dstcol_sb = const.tile([P, tot_k], f32)
        nc.sync.dma_start(out=dstcol_sb[:], in_=dstcol_in[:])

        # ---- rsqrt(deg_out) (global, p-major) --------------------------
        # one overlapped load: partition p gets rp_src[p*rows_pp : p*rows_pp+rows_pp+1]
        rpo = ph1.tile([P, rows_pp + 1], i32, tag="rp")
        nc.sync.dma_start(
            out=rpo[:],
            in_=bass.AP(tensor=rp_src[:].tensor, offset=0, ap=[[rows_pp, P], [1, rows_pp + 1]]),
        )
        deg_i = ph1.tile([P, rows_pp], i32, tag="degi")
        nc.vector.tensor_tensor(
            out=deg_i[:], in0=rpo[:, 1 : rows_pp + 1], in1=rpo[:, 0:rows_pp], op=Alu.subtract
        )
        dgo = const.tile([P, rows_pp], f32)  # persists through phase 1
        nc.vector.tensor_copy(out=dgo[:], in_=deg_i[:])
        nc.vector.tensor_scalar_max(out=dgo[:], in0=dgo[:], scalar1=1.0)
        nc.scalar.sqrt(out=dgo[:], in_=dgo[:])
        nc.vector.reciprocal(out=dgo[:], in_=dgo[:])
        if debug:
            nc.sync.dma_start(out=dbg_deg[:], in_=dgo[:])

        # ---- rsqrt(deg_in) for my nodes -> din_scr (node order) --------
        rpdo = ph1.tile([P, nblk + 1], i32, tag="rpd")
        nc.sync.dma_start(
            out=rpdo[:],
            in_=bass.AP(tensor=rp_dst_mine[:].tensor, offset=0, ap=[[nblk, P], [1, nblk + 1]]),
        )
        din_i = ph1.tile([P, nblk], i32, tag="dini")
        nc.vector.tensor_tensor(
            out=din_i[:], in0=rpdo[:, 1 : nblk + 1], in1=rpdo[:, 0:nblk], op=Alu.subtract
        )
        din_f = ph1.tile([P, nblk], f32, tag="dinf")
        nc.vector.tensor_copy(out=din_f[:], in_=din_i[:])
        nc.vector.tensor_scalar_max(out=din_f[:], in0=din_f[:], scalar1=1.0)
        nc.scalar.sqrt(out=din_f[:], in_=din_f[:])
        nc.vector.reciprocal(out=din_f[:], in_=din_f[:])
        nc.sync.dma_start(out=din_scr[:].rearrange("(p r) -> p r", p=P), in_=din_f[:])
        if debug:
            nc.sync.dma_start(out=dbg_din[:].rearrange("(p r) -> p r", p=P), in_=din_f[:])

        # ---- phase 1: h table = fp16(feats * rsqrt(deg_out)) -----------
        fview = feats_pad[:].rearrange("(p r) d -> p r d", p=P)
        hview = h_dram[:].rearrange("(p r) d -> p r d", p=P)
        n_chunks = 8
        cw = -(-rows_pp // n_chunks)
        for c in range(n_chunks):
            r0 = c * cw
            r1 = min(r0 + cw, rows_pp)
            if r0 >= r1:
                break
            w_ = r1 - r0
            ft = ph1.tile([P, cw, D], f32, tag="ft")
            nc.sync.dma_start(out=ft[:, :w_, :], in_=fview[:, r0:r1, :])
            ht = ph1.tile([P, cw, D], f16, tag="ht")
            for r in range(w_):
                nc.vector.tensor_scalar(
                    out=ht[:, r, :],
                    in0=ft[:, r, :],
                    scalar1=dgo[:, r0 + r : r0 + r + 1],
                    scalar2=None,
                    op0=Alu.mult,
                )
            nc.sync.dma_start(out=hview[:, r0:r1, :], in_=ht[:, :w_, :])
            if debug:
                nc.sync.dma_start(
                    out=dbg_h[:].rearrange("(p r) d -> p r d", p=P)[:, r0:r1, :],
                    in_=ht[:, :w_, :],
                )

        tc.strict_bb_all_engine_barrier()

        # ---- phase 2: per-block aggregation + epilogue -----------------
        for j in range(nblk):
            kj = K[j]
            bs = min(P, npc - j * P)
            agg_ps = pp.tile([D, P], f32, tag="agg")
            for k in range(kj):
                g = gp.tile([P, D], f16, tag="g")
                nc.gpsimd.indirect_dma_start(
                    out=g[:],
                    out_offset=None,
                    in_=h_dram[:],
                    in_offset=bass.IndirectOffsetOnAxis(
                        ap=gidx_sb[:, C[j] + k : C[j] + k + 1], axis=0
                    ),
                )
                if debug and j == 0:
                    nc.sync.dma_start(out=dbg_g[:, k * D : (k + 1) * D], in_=g[:])
                ind = indp.tile([P, P], f16, tag="ind")
                nc.vector.tensor_scalar(
                    out=ind[:],
                    in0=iota_sb[:],
                    scalar1=dstcol_sb[:, C[j] + k : C[j] + k + 1],
                    scalar2=None,
                    op0=Alu.is_equal,
                )
                nc.tensor.matmul(
                    out=agg_ps[:],
                    lhsT=g[:],
                    rhs=ind[:],
                    start=(k == 0),
                    stop=(k == kj - 1),
                )
            agg_sb = ep.tile([D, P], f16, tag="aggsb")
            nc.vector.tensor_copy(out=agg_sb[:], in_=agg_ps[:])
            if debug and j == 0:
                nc.sync.dma_start(out=dbg_agg[:], in_=agg_sb[:])
            w_ps = pp.tile([D, P], f32, tag="wps")
            nc.tensor.matmul(out=w_ps[:], lhsT=w_sb[:], rhs=agg_sb[:], start=True, stop=True)
            w_sbuf = ep.tile([D, P], f32, tag="wsb")
            nc.vector.tensor_copy(out=w_sbuf[:], in_=w_ps[:])
            t_ps = pp.tile([P, D], f32, tag="tps")
            nc.tensor.transpose(out=t_ps[:], in_=w_sbuf[:], identity=ident_sb[:])

            din = ep.tile([P, 1], f32, tag="din")
            nc.sync.dma_start(out=din[:], in_=din_scr[j * P : (j + 1) * P, None])
            x = ep.tile([P, D], f32, tag="x")
            nc.vector.tensor_scalar(
                out=x[:], in0=t_ps[:], scalar1=din[:], scalar2=None, op0=Alu.mult
            )
            nc.vector.tensor_tensor(out=x[:], in0=x[:], in1=b_bc[:], op=Alu.add)
            nc.scalar.activation(out=x[:], in_=x[:], func=Act.Relu)
            f = ep.tile([P, D], f32, tag="f")
            nc.sync.dma_start(out=f[:bs, :], in_=feats_mine[j * P : j * P + bs, :])
            nc.vector.tensor_tensor(out=x[:bs, :], in0=x[:bs, :], in1=f[:bs, :], op=Alu.add)
            stats = ep.tile([P, 6], f32, tag="st")
            nc.vector.bn_stats(out=stats[:bs, :], in_=x[:bs, :])
            mv = ep.tile([P, 2], f32, tag="mv")
            nc.vector.bn_aggr(out=mv[:bs, :], in_=stats[:bs, :])
            sd = ep.tile([P, 1], f32, tag="sd")
            nc.scalar.activation(
                out=sd[:bs, :], in_=mv[:bs, 1:2], func=Act.Sqrt, bias=eps_sb[:bs, :]
            )
            nc.vector.reciprocal(out=sd[:bs, :], in_=sd[:bs, :])
            y = ep.tile([P, D], f32, tag="y")
            nc.vector.tensor_scalar(
                out=y[:bs, :],
                in0=x[:bs, :],
                scalar1=mv[:bs, 0:1],
                scalar2=sd[:bs, :],
                op0=Alu.subtract,
                op1=Alu.mult,
            )
            nc.vector.tensor_tensor(out=y[:bs, :], in0=y[:bs, :], in1=gamma_bc[:bs, :], op=Alu.mult)
            nc.vector.tensor_tensor(out=y[:bs, :], in0=y[:bs, :], in1=beta_bc[:bs, :], op=Alu.add)
            nc.sync.dma_start(out=out_t[j * P : j * P + bs, :], in_=y[:bs, :])

    if split_waits:
        _split_multiwaits(nc, mybir)
    return nc


# ---------------------------------------------------------------- entry point
def kernel(feats, src, dst, W, b, gamma, beta):
    _ensure_path()
    from concourse.bass_utils import run_bass_kernel_spmd

    n_cores = 8
    feats = np.asarray(feats, np.float32)
    in_maps, meta = host_prep(feats, src, dst, W, b, gamma, beta, n_cores)
    nc = build_nc(meta)
    res = run_bass_kernel_spmd(nc, in_maps, core_ids=list(range(n_cores)))
    out = np.concatenate([r["out"] for r in res.results], axis=0)
    return out[: meta["N"]].astype(np.float32)

